# revision 25
# baseline (speedup 1.0000x reference)
"""MoE (8-expert top-2 SwiGLU + shared MLP) Trainium2 kernel, 8-core data-parallel.

Data-parallel over the 8192 tokens (1024/core, no collectives). Each core:
router (fp16 matmul + fp32 softmax + top-2 via top-8 sort), positions via
triangular-ones matmul cumsum, token dispatch via an inverse-permutation
scatter + SWDGE dma_gather(transpose=True) that lands x_e^T directly in SBUF,
SwiGLU expert GEMMs over capacity-padded token batches, and a final combine
via indirect row gather of each token's two expert outputs, fused with the
shared-MLP GEMM2 so the PE stays busy through the tail.

vs the 836us baseline:
- dispatch mask-matmuls (66us of PE time) replaced by dma_gather(transpose=True)
  from DRAM x rows using a scattered slot->token index table (DMA, hidden
  under the shared-MLP GEMM1).
- per-expert capacities computed from host-side routing of the actual input
  (max over cores + slack) instead of one global CAP.
- router runs from the resident fp16 x^T; shared-MLP GEMM1 emitted right after
  the router matmuls so the PE streams it while the softmax/top-2/positions
  vector chain runs; positions emitted mid-GEMM1; shared-MLP GEMM2 fused into
  the final combine so the PE stays busy through the gather tail.
- x^T / router weights host-packed to the SBUF layout for line-rate DMA;
  weight loads on the sync HWDGE queue, ybuf/out stores on the scalar queue.
The 2/3 (moe) and 1/3 (shared) output scales are folded into w2/ws2 on host.
"""

import os
import sys
import numpy as np

sys.path.insert(0, "/opt/trn_rl_repo")

import ml_dtypes  # noqa: E402
from concourse import bacc, mybir  # noqa: E402
from concourse.bass import IndirectOffsetOnAxis  # noqa: E402
from concourse.tile import TileContext  # noqa: E402
from concourse.bass_utils import run_bass_kernel_spmd  # noqa: E402

F32 = mybir.dt.float32
F32R = mybir.dt.float32r
I32 = mybir.dt.int32
I16 = mybir.dt.int16
BF16 = mybir.dt.bfloat16
AF = mybir.ActivationFunctionType
OP = mybir.AluOpType

DT_NAME = os.environ.get("KERNEL_DT", "fp16")
F16 = mybir.dt.float16
DT = {"f32r": F32R, "bf16": BF16, "fp16": F16}[DT_NAME]
NP_DT = {"f32r": np.float32, "bf16": ml_dtypes.bfloat16, "fp16": np.float16}[DT_NAME]

D = 1024
E = 8
HID = 2048
SH = 2048
NCORES = 8
T = 8192
TC = T // NCORES
NTT = TC // 128   # 8 token tiles / core
NDC = D // 128    # 8
NHC = HID // 128  # 16
CAPPAD = 384      # slot->token table stride (dma_gather needs %128 idxs)
NIC = CAPPAD // 16  # idx columns per expert in the wrapped int16 layout
CSLACK = 4        # per-expert capacity slack over host-measured max count
DW = 512 if DT in (BF16, F16) else 256   # GEMM2 moving width
NDQ = D // DW

_PROGRAMS = {}


def _build_program(caps):
    caps = list(caps)
    CAPMAX = max(caps)
    ybase = [0] * E
    for e in range(1, E):
        ybase[e] = ybase[e - 1] + caps[e - 1]
    YR = ybase[-1] + caps[-1]

    nc = bacc.Bacc()

    x_tok = nc.declare_dram_parameter("x_tok", [TC, D], DT, isOutput=False)
    x_trp = nc.declare_dram_parameter("x_trp", [128, NDC, TC], DT, isOutput=False)
    wrp = nc.declare_dram_parameter("wrp", [128, NDC, E], DT, isOutput=False)
    # packed weights (see kernel() for host-side layouts)
    w1p = nc.declare_dram_parameter("w1p", [E, 8, 128, NDC, 256], DT, isOutput=False)
    w3p = nc.declare_dram_parameter("w3p", [E, 8, 128, NDC, 256], DT, isOutput=False)
    w2p = nc.declare_dram_parameter("w2p", [E, NDQ, 2, 128, 8, DW], DT, isOutput=False)
    ws1p = nc.declare_dram_parameter("ws1p", [8, 128, NDC, 256], DT, isOutput=False)
    ws3p = nc.declare_dram_parameter("ws3p", [8, 128, NDC, 256], DT, isOutput=False)
    ws2p = nc.declare_dram_parameter("ws2p", [NDQ, 128, NHC, DW], DT, isOutput=False)
    cpack = nc.declare_dram_parameter("cpack", [128, 272], F32, isOutput=False)
    tok16 = nc.declare_dram_parameter("tok16", [128, NTT, 16], I16, isOutput=False)
    out = nc.declare_dram_parameter("out", [TC, D], F32, isOutput=True)

    ybufs = [nc.dram_tensor(f"ybuf{q}", [YR, DW], F32) for q in range(NDQ)]
    # slot->token table: [e, col, s, replica] so row (e*CAPPAD + col*16 + s)
    # holds 16 int16 replicas of the token index for slot col*16+s
    inv16 = nc.dram_tensor("inv16", [E, NIC, 16, 16], I16)
    inv_rows = inv16.rearrange("e c s r -> (e c s) r")

    with TileContext(nc) as tc:
        with (
            tc.tile_pool(name="const", bufs=1) as cpool,
            tc.tile_pool(name="route", bufs=1) as rpool,
            tc.tile_pool(name="big", bufs=1) as bpool,
            tc.tile_pool(name="wts", bufs=2) as wpool,
            tc.tile_pool(name="work", bufs=2) as kpool,
            tc.tile_pool(name="ps_small", bufs=2, space="PSUM") as ps_s,
            tc.tile_pool(name="ps_uv", bufs=1, space="PSUM") as ps_uv,
            tc.tile_pool(name="ps_y", bufs=4, space="PSUM") as ps_y,
        ):
            # ---- HAM warm-up: dummy matmuls on a memset tile while the
            # first DMAs are in flight, so the PE clock is at 2.4GHz when
            # real work arrives. Result sunk to DRAM to survive DCE.
            warm_sink = nc.dram_tensor("warm_sink", [128, 512], F32)
            wdum = cpool.tile([128, 512], DT, tag="wdum")
            nc.vector.memset(wdum[:], 0)
            psd = ps_y.tile([128, 512], F32, tag="psy", name="psd_warm")
            for i in range(16):
                nc.tensor.matmul(psd[:], wdum[:, :128], wdum[:],
                                 start=(i == 0), stop=(i == 15))
            wsb = kpool.tile([128, 512], F32, tag="ysb")
            nc.scalar.copy(wsb[:], psd[:])
            nc.scalar.dma_start(out=warm_sink[:, :], in_=wsb[:])
            # dummy dma_gather so the gpsimd mlp ucode library loads now,
            # during the startup DMA wait, instead of lazily right before
            # the first expert gather (measured 9.3us on that critical path)
            warm_sink2 = nc.dram_tensor("warm_sink2", [128, 8], DT)
            z8 = cpool.tile([128, 8], I16, tag="z8")
            nc.vector.memset(z8[:], 0)
            dxe = kpool.tile([128, NDC, 128], DT, tag="xe", bufs=2)
            nc.gpsimd.dma_gather(
                out_ap=dxe[:], in_ap=x_tok[:, :], idxs_ap=z8[:],
                num_idxs=128, num_idxs_reg=128, elem_size=D, transpose=True)
            nc.sync.dma_start(out=warm_sink2[:, :], in_=dxe[:, 0, 0:8])

            # ---- x^T (host-packed, line-rate) + first weights on sync -----
            xtr_t = bpool.tile([128, NDC, TC], DT, tag="xbig")
            nc.sync.dma_start(out=xtr_t[:], in_=x_trp[:])
            wr_t = cpool.tile([128, NDC, E], DT, tag="wr")
            nc.sync.dma_start(out=wr_t[:], in_=wrp[:])
            sw1_0 = wpool.tile([128, NDC, 256], DT, tag="w1q", bufs=3, name="sw1_0")
            nc.sync.dma_start(out=sw1_0[:], in_=ws1p[0])
            sw3_0 = wpool.tile([128, NDC, 256], DT, tag="w3q", bufs=3, name="sw3_0")
            nc.sync.dma_start(out=sw3_0[:], in_=ws3p[0])

            # ---- resident constants (one packed DMA) ----------------------
            cpk = cpool.tile([128, 272], F32, tag="cpack")
            nc.sync.dma_start(out=cpk[:], in_=cpack[:])
            uts_t = cpk[:, 0:128]
            ones_t = cpk[:, 128:256]
            ecap_t = cpk[:, 256:264]
            ecap2_t = cpk[:, 264:272]
            tok16_t = cpool.tile([128, NTT, 16], I16, tag="tok16")
            nc.sync.dma_start(out=tok16_t[:], in_=tok16[:])

            mask_all = rpool.tile([128, NTT, E], F32, tag="mask")
            m1_all = rpool.tile([128, NTT, E], F32, tag="m1")
            t8_all = rpool.tile([128, NTT, 8], F32, tag="t8")
            off_all = rpool.tile([128, NTT, 2], I32, tag="off")
            off2_all = rpool.tile([128, NTT, 2], I32, tag="off2")
            lgacc = rpool.tile([128, NTT, E], F32, tag="lgacc")

            # ---- Router matmuls (fp16 x^T resident) -----------------------
            for tt in range(NTT):
                ps_l = ps_s.tile([128, E], F32, tag="small")
                for dc in range(NDC):
                    nc.tensor.matmul(
                        ps_l[:],
                        xtr_t[:, dc, tt * 128:(tt + 1) * 128],
                        wr_t[:, dc, :],
                        start=(dc == 0), stop=(dc == NDC - 1),
                    )
                nc.scalar.copy(lgacc[:, tt, :], ps_l[:])

            # zero the slot->token table (unrouted slots then gather token 0)
            z16 = cpool.tile([128, E * NIC * 2], I16, tag="z16")
            nc.vector.memset(z16[:], 0)
            nc.sync.dma_start(out=inv16.rearrange("e c s r -> (e c s r)")
                              .rearrange("(p q) -> p q", p=128), in_=z16[:])

            # ---- softmax + top-2 (emitted before shared GEMM1 so its Exp
            # ops sit ahead of the 128 silu ACTIVATEs in the scalar FIFO) ---
            for tt in range(NTT):
                lg = lgacc[:, tt, :]
                negmx = rpool.tile([128, 1], F32, tag="negmx")
                nc.vector.reduce_max(negmx[:], lg[:], axis=mybir.AxisListType.X,
                                     negate=True)
                ex = rpool.tile([128, E], F32, tag="ex")
                sm = rpool.tile([128, 1], F32, tag="sm")
                nc.scalar.activation(ex[:], lg[:], AF.Exp, bias=negmx[:],
                                     scale=1.0, accum_out=sm[:])
                rcp = rpool.tile([128, 1], F32, tag="rcp")
                nc.vector.reciprocal(rcp[:], sm[:])
                probs = rpool.tile([128, E], F32, tag="probs")
                nc.vector.tensor_scalar_mul(probs[:], ex[:], rcp[:])
                nc.vector.max(t8_all[:, tt, :], probs[:])
                nc.vector.tensor_tensor(
                    out=m1_all[:, tt, :], in0=probs[:],
                    in1=t8_all[:, tt, 0:1].to_broadcast([128, E]),
                    op=OP.is_ge)
                nc.vector.tensor_tensor(
                    out=mask_all[:, tt, :], in0=probs[:],
                    in1=t8_all[:, tt, 1:2].to_broadcast([128, E]),
                    op=OP.is_ge)

            def emit_positions_and_dispatch():
                # positions (cumsum over token tiles), gather offsets:
                # off  = pos + cumulative-cap base (ybuf row of the token)
                # off2 = pos + e*CAPPAD          (inv16 row of the token)
                for tt in range(NTT):
                    ps_p = ps_s.tile([128, E], F32, tag="small")
                    for tp in range(tt):
                        nc.tensor.matmul(ps_p[:], ones_t, mask_all[:, tp, :],
                                         start=(tp == 0), stop=False)
                    nc.tensor.matmul(ps_p[:], uts_t, mask_all[:, tt, :],
                                     start=(tt == 0), stop=True)
                    m2 = rpool.tile([128, E], F32, tag="m2")
                    nc.vector.tensor_sub(m2[:], mask_all[:, tt, :],
                                         m1_all[:, tt, :])
                    for cst, offt in ((ecap_t, off_all), (ecap2_t, off2_all)):
                        sl = rpool.tile([128, E], F32, tag="sl")
                        nc.vector.tensor_add(sl[:], ps_p[:], cst)
                        s1m = rpool.tile([128, E], F32, tag="s1m")
                        nc.vector.tensor_mul(s1m[:], sl[:], m1_all[:, tt, :])
                        s1f = rpool.tile([128, 1], F32, tag="s1f")
                        nc.vector.reduce_sum(s1f[:], s1m[:],
                                             axis=mybir.AxisListType.X)
                        nc.vector.tensor_copy(offt[:, tt, 0:1], s1f[:])
                        s2m = rpool.tile([128, E], F32, tag="s2m")
                        nc.vector.tensor_mul(s2m[:], sl[:], m2[:])
                        s2f = rpool.tile([128, 1], F32, tag="s2f")
                        nc.vector.reduce_sum(s2f[:], s2m[:],
                                             axis=mybir.AxisListType.X)
                        nc.vector.tensor_copy(offt[:, tt, 1:2], s2f[:])

                # scatter token ids into the slot->token table (gpsimd queue)
                for tt in range(NTT):
                    for k in range(2):
                        nc.gpsimd.indirect_dma_start(
                            out=inv_rows[:, :], out_offset=IndirectOffsetOnAxis(
                                ap=off2_all[:, tt, k:k + 1], axis=0),
                            in_=tok16_t[:, tt, :], in_offset=None)
                # wrapped int16 idx tiles: partition r*16+s, col (e, c) =
                # token of slot c*16+s (replicated for the Q7 cores); on the
                # gpsimd queue so the waits don't block sync weight loads
                for r in range(8):
                    nc.gpsimd.dma_start(out=it_all[r * 16:(r + 1) * 16, :, :],
                                        in_=inv16.transpose((2, 3, 0, 1))[:, r])

            it_all = cpool.tile([128, E, NIC], I16, tag="idx")

            # ---- Shared MLP GEMM1 into resident gs_full -------------------
            # (PE streams this while the softmax/positions/scatter/gather
            # routing chain runs on the other engines)
            gs_full = bpool.tile([128, NHC, TC], DT, tag="gshared")
            for hqg in range(8):
                if hqg == 0:
                    wq1, wq3 = sw1_0, sw3_0
                else:
                    wq1 = wpool.tile([128, NDC, 256], DT, tag="w1q", bufs=3)
                    nc.sync.dma_start(out=wq1[:], in_=ws1p[hqg])
                    wq3 = wpool.tile([128, NDC, 256], DT, tag="w3q", bufs=3)
                    nc.sync.dma_start(out=wq3[:], in_=ws3p[hqg])
                for ht in range(2):
                    hg = hqg * 2 + ht
                    for ts in range(2):
                        psu = ps_uv.tile([128, 512], F32, tag="psu")
                        psv = ps_uv.tile([128, 512], F32, tag="psv")
                        for dc in range(NDC):
                            nc.tensor.matmul(
                                psu[:],
                                wq1[:, dc, ht * 128:(ht + 1) * 128],
                                xtr_t[:, dc, ts * 512:(ts + 1) * 512],
                                start=(dc == 0), stop=(dc == NDC - 1))
                        for dc in range(NDC):
                            nc.tensor.matmul(
                                psv[:],
                                wq3[:, dc, ht * 128:(ht + 1) * 128],
                                xtr_t[:, dc, ts * 512:(ts + 1) * 512],
                                start=(dc == 0), stop=(dc == NDC - 1))
                        su = kpool.tile([128, 512], F32, tag="su")
                        nc.scalar.activation(su[:], psu[:], AF.Silu)
                        nc.vector.tensor_mul(
                            gs_full[:, hg, ts * 512:(ts + 1) * 512],
                            su[:], psv[:])
                if hqg == 0:
                    # mask_all is ready by now; run the routing chain so the
                    # expert gathers complete long before the expert GEMMs
                    emit_positions_and_dispatch()

            out_v = out.rearrange("(tt p) d -> p tt d", p=128)

            # shared GEMM2 weights for the fused combine
            w2s_tiles = [wpool.tile([128, NHC, DW], DT, tag="w2s", bufs=2,
                                    name=f"w2s_{dq}") for dq in range(NDQ)]

            def emit_combine(dq):
                # Combine fused with shared GEMM2: PE computes the shared-MLP
                # contribution per tt while the indirect gathers of the two
                # expert rows run on the DMA engines. Interleaved after each
                # d-half's expert GEMM2 so dq0's gathers and vector work
                # overlap dq1's GEMM2; out stores ride the then-idle sync
                # queue so they never block the ybuf store triggers.
                if dq + 1 < NDQ:
                    nc.scalar.dma_start(out=w2s_tiles[dq + 1][:],
                                        in_=ws2p[dq + 1])
                w2s = w2s_tiles[dq]
                for tt in range(NTT):
                    psy = ps_y.tile([128, DW], F32, tag="psy")
                    for hc in range(NHC):
                        nc.tensor.matmul(
                            psy[:],
                            gs_full[:, hc, tt * 128:(tt + 1) * 128],
                            w2s[:, hc, :],
                            start=(hc == 0), stop=(hc == NHC - 1))
                    y1 = kpool.tile([128, DW], F32, tag="late", bufs=3)
                    nc.gpsimd.indirect_dma_start(
                        out=y1[:], out_offset=None,
                        in_=ybufs[dq][:, :],
                        in_offset=IndirectOffsetOnAxis(
                            ap=off_all[:, tt, 0:1], axis=0))
                    y2 = kpool.tile([128, DW], F32, tag="late2", bufs=3)
                    nc.gpsimd.indirect_dma_start(
                        out=y2[:], out_offset=None,
                        in_=ybufs[dq][:, :],
                        in_offset=IndirectOffsetOnAxis(
                            ap=off_all[:, tt, 1:2], axis=0))
                    fin = kpool.tile([128, DW], F32, tag="fin", bufs=3)
                    nc.vector.tensor_scalar_mul(
                        fin[:], y1[:], scalar1=t8_all[:, tt, 0:1])
                    y2s = kpool.tile([128, DW], F32, tag="y2s", bufs=3)
                    nc.scalar.activation(y2s[:], y2[:], AF.Copy,
                                         scale=t8_all[:, tt, 1:2])
                    nc.vector.tensor_add(fin[:], fin[:], y2s[:])
                    nc.vector.tensor_add(fin[:], fin[:], psy[:])
                    nc.sync.dma_start(
                        out=out_v[:, tt, dq * DW:(dq + 1) * DW],
                        in_=fin[:])

            # ---- Experts: two halves of 4 ---------------------------------
            EH = E // 2
            for half in range(2):
                g_all = bpool.tile([128, EH, NHC, CAPMAX], DT, tag="g",
                                   name=f"g_all_{half}")
                for ei in range(EH):
                    e = half * EH + ei
                    ce = caps[e]
                    xe_t = kpool.tile([128, NDC, CAPPAD], DT, tag="xe", bufs=2)
                    nc.gpsimd.dma_gather(
                        out_ap=xe_t[:], in_ap=x_tok[:, :],
                        idxs_ap=it_all[:, e, :],
                        num_idxs=CAPPAD, num_idxs_reg=CAPPAD,
                        elem_size=D, transpose=True)

                    for hq in range(8):
                        wq1 = wpool.tile([128, NDC, 256], DT, tag="w1q", bufs=3)
                        nc.sync.dma_start(out=wq1[:], in_=w1p[e, hq])
                        wq3 = wpool.tile([128, NDC, 256], DT, tag="w3q", bufs=3)
                        nc.sync.dma_start(out=wq3[:], in_=w3p[e, hq])
                        for ht in range(2):
                            hg = hq * 2 + ht
                            psu = ps_uv.tile([128, CAPMAX], F32, tag="psu")
                            psv = ps_uv.tile([128, CAPMAX], F32, tag="psv")
                            for dc in range(NDC):
                                nc.tensor.matmul(
                                    psu[:, :ce],
                                    wq1[:, dc, ht * 128:(ht + 1) * 128],
                                    xe_t[:, dc, :ce],
                                    start=(dc == 0), stop=(dc == NDC - 1))
                            for dc in range(NDC):
                                nc.tensor.matmul(
                                    psv[:, :ce],
                                    wq3[:, dc, ht * 128:(ht + 1) * 128],
                                    xe_t[:, dc, :ce],
                                    start=(dc == 0), stop=(dc == NDC - 1))
                            su = kpool.tile([128, CAPMAX], F32, tag="su")
                            nc.scalar.activation(su[:, :ce], psu[:, :ce], AF.Silu)
                            nc.vector.tensor_mul(g_all[:, ei, hg, :ce],
                                                 su[:, :ce], psv[:, :ce])

                # prefetch shared GEMM2 weights while expert GEMM2 runs
                if half == 1:
                    nc.scalar.dma_start(out=w2s_tiles[0][:], in_=ws2p[0])

                # GEMM2 for this half's 4 experts, d-half (dq) outer
                for dq in range(NDQ):
                    for ei in range(EH):
                        e = half * EH + ei
                        ce = caps[e]
                        nct = (ce + 127) // 128
                        psy_l = [ps_y.tile([128, DW], F32, tag="psy",
                                           name=f"psy_{e}_{dq}_{i}")
                                 for i in range(nct)]
                        for qh in range(2):
                            w2q = wpool.tile([128, 8, DW], DT, tag="w2q")
                            nc.sync.dma_start(out=w2q[:], in_=w2p[e, dq, qh])
                            for ct in range(nct):
                                cw = min(128, ce - ct * 128)
                                for hc in range(8):
                                    nc.tensor.matmul(
                                        psy_l[ct][:cw],
                                        g_all[:, ei, qh * 8 + hc,
                                              ct * 128:ct * 128 + cw],
                                        w2q[:, hc, :],
                                        start=(qh == 0 and hc == 0),
                                        stop=(qh == 1 and hc == 7))
                        for ct in range(nct):
                            cw = min(128, ce - ct * 128)
                            ysb = kpool.tile([128, DW], F32, tag="ysb")
                            nc.vector.tensor_copy(ysb[:cw], psy_l[ct][:cw])
                            nc.scalar.dma_start(
                                out=ybufs[dq][ybase[e] + ct * 128:
                                              ybase[e] + ct * 128 + cw, :],
                                in_=ysb[:cw])
                    if half == 1:
                        emit_combine(dq)


    nc.finalize()
    return nc


def _get_program(caps):
    key = tuple(caps)
    if key not in _PROGRAMS:
        _PROGRAMS[key] = _build_program(key)
    return _PROGRAMS[key]


def _pack_w13(w):
    # [E, D, HID] -> [E, hq, p, dc, col] so each (e,hq) load is contiguous
    return np.ascontiguousarray(
        w.reshape(E, NDC, 128, 8, 256).transpose(0, 3, 2, 1, 4).astype(NP_DT))


def _pack_w2(w):
    # [E, HID, D] -> [E, dq, qh, p, hcl, col]
    return np.ascontiguousarray(
        w.reshape(E, 2, 8, 128, NDQ, DW).transpose(0, 4, 1, 3, 2, 5).astype(NP_DT))


def _pack_ws13(w):
    # [D, SH] -> [hqg, p, dc, col]
    return np.ascontiguousarray(
        w.reshape(NDC, 128, 8, 256).transpose(2, 1, 0, 3).astype(NP_DT))


def _pack_ws2(w):
    # [SH, D] -> [dq, p, hc, col]
    return np.ascontiguousarray(
        w.reshape(NHC, 128, NDQ, DW).transpose(2, 1, 0, 3).astype(NP_DT))


def _routing_caps(xf, w_router):
    """Per-expert capacity: max per-core count from fp32 routing + slack."""
    logits = xf @ w_router
    part = np.argpartition(-logits, 2, axis=1)[:, :2]
    mask = np.zeros(logits.shape, bool)
    mask[np.arange(len(xf))[:, None], part] = True
    cnt = mask.reshape(NCORES, TC, E).sum(axis=1)
    return [int(c) + CSLACK for c in cnt.max(axis=0)]


def kernel(x, w_router, w1, w3, w2, ws1, ws3, ws2):
    x = np.asarray(x, dtype=np.float32)
    w_router = np.ascontiguousarray(np.asarray(w_router, dtype=np.float32))
    w1 = np.asarray(w1, dtype=np.float32)
    w3 = np.asarray(w3, dtype=np.float32)
    w2 = np.asarray(w2, dtype=np.float32) * (2.0 / 3.0)
    ws1 = np.asarray(ws1, dtype=np.float32)
    ws3 = np.asarray(ws3, dtype=np.float32)
    ws2 = np.asarray(ws2, dtype=np.float32) * (1.0 / 3.0)

    orig_shape = x.shape
    xf = np.ascontiguousarray(x.reshape(T, D))

    caps = _routing_caps(xf, w_router)
    ybase = np.concatenate([[0], np.cumsum(caps)[:-1]]).astype(np.float32)

    idx = np.arange(128, dtype=np.float32)
    uts = (idx[:, None] < idx[None, :]).astype(np.float32)
    ones = np.ones((128, 128), dtype=np.float32)
    ecap = np.broadcast_to(ybase, (128, E))
    ecap2 = np.broadcast_to(
        np.arange(E, dtype=np.float32) * CAPPAD, (128, E))
    cpack = np.ascontiguousarray(
        np.concatenate([uts, ones, ecap, ecap2], axis=1, dtype=np.float32))
    tok = (np.arange(TC, dtype=np.int16).reshape(NTT, 128).T)[:, :, None]
    tok16 = np.ascontiguousarray(np.broadcast_to(tok, (128, NTT, 16)))

    w1p, w3p = _pack_w13(w1), _pack_w13(w3)
    w2p = _pack_w2(w2)
    ws1p, ws3p = _pack_ws13(ws1), _pack_ws13(ws3)
    ws2p = _pack_ws2(ws2)
    wrp_h = np.ascontiguousarray(
        w_router.reshape(NDC, 128, E).transpose(1, 0, 2).astype(NP_DT))

    nc = _get_program(caps)

    in_maps = []
    for c in range(NCORES):
        xc = np.ascontiguousarray(xf[c * TC:(c + 1) * TC])
        xtrp = np.ascontiguousarray(
            xc.T.reshape(NDC, 128, TC).transpose(1, 0, 2).astype(NP_DT))
        in_maps.append({
            "x_tok": xc.astype(NP_DT), "x_trp": xtrp,
            "wrp": wrp_h,
            "w1p": w1p, "w3p": w3p, "w2p": w2p,
            "ws1p": ws1p, "ws3p": ws3p, "ws2p": ws2p,
            "cpack": cpack, "tok16": tok16,
        })

    res = run_bass_kernel_spmd(nc, in_maps, list(range(NCORES)))
    out = np.concatenate([res.results[c]["out"] for c in range(NCORES)], axis=0)
    return out.reshape(orig_shape).astype(np.float32)


# revision 28
# speedup vs baseline: 1.1480x; 1.1480x over previous
"""MoE (8-expert top-2 SwiGLU + shared MLP) Trainium2 kernel, 8-core data-parallel.

Data-parallel over the 8192 tokens (1024/core, no collectives). Each core:
router (fp16 matmul + fp32 softmax + top-2 via top-8 sort), positions via
triangular-ones matmul cumsum, token dispatch via an inverse-permutation
scatter + SWDGE dma_gather(transpose=True) that lands x_e^T directly in SBUF,
SwiGLU expert GEMMs over capacity-padded token batches, and a final combine
via indirect row gather of each token's two expert outputs, fused with the
shared-MLP GEMM2 so the PE stays busy through the tail.

vs the 836us baseline:
- dispatch mask-matmuls (66us of PE time) replaced by dma_gather(transpose=True)
  from DRAM x rows using a scattered slot->token index table (DMA, hidden
  under the shared-MLP GEMM1).
- per-expert capacities computed from host-side routing of the actual input
  (max over cores + slack) instead of one global CAP.
- router runs from the resident fp16 x^T; shared-MLP GEMM1 emitted right after
  the router matmuls so the PE streams it while the softmax/top-2/positions
  vector chain runs; positions emitted mid-GEMM1; shared-MLP GEMM2 fused into
  the final combine so the PE stays busy through the gather tail.
- x^T / router weights host-packed to the SBUF layout for line-rate DMA;
  weight loads on the sync HWDGE queue, ybuf/out stores on the scalar queue.
The 2/3 (moe) and 1/3 (shared) output scales are folded into w2/ws2 on host.
"""

import os
import sys
import numpy as np

sys.path.insert(0, "/opt/trn_rl_repo")

import ml_dtypes  # noqa: E402
from concourse import bacc, mybir  # noqa: E402
from concourse.bass import IndirectOffsetOnAxis  # noqa: E402
from concourse.tile import TileContext  # noqa: E402
from concourse.bass_utils import run_bass_kernel_spmd  # noqa: E402

F32 = mybir.dt.float32
F32R = mybir.dt.float32r
I32 = mybir.dt.int32
I16 = mybir.dt.int16
BF16 = mybir.dt.bfloat16
AF = mybir.ActivationFunctionType
OP = mybir.AluOpType

DT_NAME = os.environ.get("KERNEL_DT", "fp16")
F16 = mybir.dt.float16
DT = {"f32r": F32R, "bf16": BF16, "fp16": F16}[DT_NAME]
NP_DT = {"f32r": np.float32, "bf16": ml_dtypes.bfloat16, "fp16": np.float16}[DT_NAME]

D = 1024
E = 8
HID = 2048
SH = 2048
NCORES = 8
T = 8192
TC = T // NCORES
NTT = TC // 128   # 8 token tiles / core
NDC = D // 128    # 8
NHC = HID // 128  # 16
CAPPAD = 384      # slot->token table stride (dma_gather needs %128 idxs)
NIC = CAPPAD // 16  # idx columns per expert in the wrapped int16 layout
CSLACK = 4        # per-expert capacity slack over host-measured max count
DW = 512 if DT in (BF16, F16) else 256   # GEMM2 moving width
NDQ = D // DW

_PROGRAMS = {}


def _build_program(caps):
    caps = list(caps)
    CAPMAX = max(caps)
    ybase = [0] * E
    for e in range(1, E):
        ybase[e] = ybase[e - 1] + caps[e - 1]
    YR = ybase[-1] + caps[-1]

    nc = bacc.Bacc()

    x_tok = nc.declare_dram_parameter("x_tok", [TC, D], DT, isOutput=False)
    x_trp = nc.declare_dram_parameter("x_trp", [128, NDC, TC], DT, isOutput=False)
    wrp = nc.declare_dram_parameter("wrp", [128, NDC, E], DT, isOutput=False)
    # packed weights (see kernel() for host-side layouts)
    w1p = nc.declare_dram_parameter("w1p", [E, 8, 128, NDC, 256], DT, isOutput=False)
    w3p = nc.declare_dram_parameter("w3p", [E, 8, 128, NDC, 256], DT, isOutput=False)
    w2p = nc.declare_dram_parameter("w2p", [E, NDQ, 2, 128, 8, DW], DT, isOutput=False)
    ws1p = nc.declare_dram_parameter("ws1p", [8, 128, NDC, 256], DT, isOutput=False)
    ws3p = nc.declare_dram_parameter("ws3p", [8, 128, NDC, 256], DT, isOutput=False)
    ws2p = nc.declare_dram_parameter("ws2p", [NDQ, 128, NHC, DW], DT, isOutput=False)
    cpack = nc.declare_dram_parameter("cpack", [128, 272], F32, isOutput=False)
    tok16 = nc.declare_dram_parameter("tok16", [128, NTT, 16], I16, isOutput=False)
    out = nc.declare_dram_parameter("out", [TC, D], F32, isOutput=True)

    ybufs = [nc.dram_tensor(f"ybuf{q}", [YR, DW], F32) for q in range(NDQ)]
    # slot->token table: [e, col, s, replica] so row (e*CAPPAD + col*16 + s)
    # holds 16 int16 replicas of the token index for slot col*16+s
    inv16 = nc.dram_tensor("inv16", [E, NIC, 16, 16], I16)
    inv_rows = inv16.rearrange("e c s r -> (e c s) r")

    with TileContext(nc) as tc:
        with (
            tc.tile_pool(name="const", bufs=1) as cpool,
            tc.tile_pool(name="route", bufs=1) as rpool,
            tc.tile_pool(name="big", bufs=1) as bpool,
            tc.tile_pool(name="wts", bufs=2) as wpool,
            tc.tile_pool(name="work", bufs=2) as kpool,
            tc.tile_pool(name="ps_small", bufs=2, space="PSUM") as ps_s,
            tc.tile_pool(name="ps_uv", bufs=1, space="PSUM") as ps_uv,
            tc.tile_pool(name="ps_y", bufs=4, space="PSUM") as ps_y,
        ):
            # ---- HAM warm-up: dummy matmuls on a memset tile while the
            # first DMAs are in flight, so the PE clock is at 2.4GHz when
            # real work arrives. Result sunk to DRAM to survive DCE.
            warm_sink = nc.dram_tensor("warm_sink", [128, 512], F32)
            wdum = cpool.tile([128, 512], DT, tag="wdum")
            nc.vector.memset(wdum[:], 0)
            psd = ps_y.tile([128, 512], F32, tag="psy", name="psd_warm")
            for i in range(16):
                nc.tensor.matmul(psd[:], wdum[:, :128], wdum[:],
                                 start=(i == 0), stop=(i == 15))
            wsb = kpool.tile([128, 512], F32, tag="ysb")
            nc.scalar.copy(wsb[:], psd[:])
            nc.scalar.dma_start(out=warm_sink[:, :], in_=wsb[:])
            # dummy dma_gather so the gpsimd mlp ucode library loads now,
            # during the startup DMA wait, instead of lazily right before
            # the first expert gather (measured 9.3us on that critical path)
            warm_sink2 = nc.dram_tensor("warm_sink2", [128, 8], DT)
            z8 = cpool.tile([128, 8], I16, tag="z8")
            nc.vector.memset(z8[:], 0)
            dxe = kpool.tile([128, NDC, 128], DT, tag="xe", bufs=2)
            nc.gpsimd.dma_gather(
                out_ap=dxe[:], in_ap=x_tok[:, :], idxs_ap=z8[:],
                num_idxs=128, num_idxs_reg=128, elem_size=D, transpose=True)
            nc.sync.dma_start(out=warm_sink2[:, :], in_=dxe[:, 0, 0:8])

            # ---- x^T (host-packed, line-rate) + first weights on sync -----
            xtr_t = bpool.tile([128, NDC, TC], DT, tag="xbig")
            nc.sync.dma_start(out=xtr_t[:], in_=x_trp[:])
            wr_t = cpool.tile([128, NDC, E], DT, tag="wr")
            nc.sync.dma_start(out=wr_t[:], in_=wrp[:])
            sw1_0 = wpool.tile([128, NDC, 256], DT, tag="w1q", bufs=3, name="sw1_0")
            nc.sync.dma_start(out=sw1_0[:], in_=ws1p[0])
            sw3_0 = wpool.tile([128, NDC, 256], DT, tag="w3q", bufs=3, name="sw3_0")
            nc.sync.dma_start(out=sw3_0[:], in_=ws3p[0])

            # ---- resident constants (one packed DMA) ----------------------
            cpk = cpool.tile([128, 272], F32, tag="cpack")
            nc.sync.dma_start(out=cpk[:], in_=cpack[:])
            uts_t = cpk[:, 0:128]
            ones_t = cpk[:, 128:256]
            ecap_t = cpk[:, 256:264]
            ecap2_t = cpk[:, 264:272]
            tok16_t = cpool.tile([128, NTT, 16], I16, tag="tok16")
            nc.sync.dma_start(out=tok16_t[:], in_=tok16[:])

            mask_all = rpool.tile([128, NTT, E], F32, tag="mask")
            m1_all = rpool.tile([128, NTT, E], F32, tag="m1")
            t8_all = rpool.tile([128, NTT, 8], F32, tag="t8")
            off_all = rpool.tile([128, NTT, 2], I32, tag="off")
            off2_all = rpool.tile([128, NTT, 2], I32, tag="off2")
            lgacc = rpool.tile([128, NTT, E], F32, tag="lgacc")

            # ---- Router matmuls (fp16 x^T resident) -----------------------
            for tt in range(NTT):
                ps_l = ps_s.tile([128, E], F32, tag="small")
                for dc in range(NDC):
                    nc.tensor.matmul(
                        ps_l[:],
                        xtr_t[:, dc, tt * 128:(tt + 1) * 128],
                        wr_t[:, dc, :],
                        start=(dc == 0), stop=(dc == NDC - 1),
                    )
                nc.scalar.copy(lgacc[:, tt, :], ps_l[:])

            # zero the slot->token table (unrouted slots then gather token 0)
            z16 = cpool.tile([128, E * NIC * 2], I16, tag="z16")
            nc.vector.memset(z16[:], 0)
            nc.sync.dma_start(out=inv16.rearrange("e c s r -> (e c s r)")
                              .rearrange("(p q) -> p q", p=128), in_=z16[:])

            # ---- softmax + top-2 (emitted before shared GEMM1 so its Exp
            # ops sit ahead of the 128 silu ACTIVATEs in the scalar FIFO) ---
            for tt in range(NTT):
                lg = lgacc[:, tt, :]
                negmx = rpool.tile([128, 1], F32, tag="negmx")
                nc.vector.reduce_max(negmx[:], lg[:], axis=mybir.AxisListType.X,
                                     negate=True)
                ex = rpool.tile([128, E], F32, tag="ex")
                sm = rpool.tile([128, 1], F32, tag="sm")
                nc.scalar.activation(ex[:], lg[:], AF.Exp, bias=negmx[:],
                                     scale=1.0, accum_out=sm[:])
                rcp = rpool.tile([128, 1], F32, tag="rcp")
                nc.vector.reciprocal(rcp[:], sm[:])
                probs = rpool.tile([128, E], F32, tag="probs")
                nc.vector.tensor_scalar_mul(probs[:], ex[:], rcp[:])
                nc.vector.max(t8_all[:, tt, :], probs[:])
                nc.vector.tensor_tensor(
                    out=m1_all[:, tt, :], in0=probs[:],
                    in1=t8_all[:, tt, 0:1].to_broadcast([128, E]),
                    op=OP.is_ge)
                nc.vector.tensor_tensor(
                    out=mask_all[:, tt, :], in0=probs[:],
                    in1=t8_all[:, tt, 1:2].to_broadcast([128, E]),
                    op=OP.is_ge)

            def emit_positions_and_dispatch():
                # positions (cumsum over token tiles), gather offsets:
                # off  = pos + cumulative-cap base (ybuf row of the token)
                # off2 = pos + e*CAPPAD          (inv16 row of the token)
                for tt in range(NTT):
                    ps_p = ps_s.tile([128, E], F32, tag="small")
                    for tp in range(tt):
                        nc.tensor.matmul(ps_p[:], ones_t, mask_all[:, tp, :],
                                         start=(tp == 0), stop=False)
                    nc.tensor.matmul(ps_p[:], uts_t, mask_all[:, tt, :],
                                     start=(tt == 0), stop=True)
                    m2 = rpool.tile([128, E], F32, tag="m2")
                    nc.vector.tensor_sub(m2[:], mask_all[:, tt, :],
                                         m1_all[:, tt, :])
                    for cst, offt in ((ecap_t, off_all), (ecap2_t, off2_all)):
                        sl = rpool.tile([128, E], F32, tag="sl")
                        nc.vector.tensor_add(sl[:], ps_p[:], cst)
                        s1m = rpool.tile([128, E], F32, tag="s1m")
                        nc.vector.tensor_mul(s1m[:], sl[:], m1_all[:, tt, :])
                        s1f = rpool.tile([128, 1], F32, tag="s1f")
                        nc.vector.reduce_sum(s1f[:], s1m[:],
                                             axis=mybir.AxisListType.X)
                        nc.vector.tensor_copy(offt[:, tt, 0:1], s1f[:])
                        s2m = rpool.tile([128, E], F32, tag="s2m")
                        nc.vector.tensor_mul(s2m[:], sl[:], m2[:])
                        s2f = rpool.tile([128, 1], F32, tag="s2f")
                        nc.vector.reduce_sum(s2f[:], s2m[:],
                                             axis=mybir.AxisListType.X)
                        nc.vector.tensor_copy(offt[:, tt, 1:2], s2f[:])

                # scatter token ids into the slot->token table (gpsimd queue)
                for tt in range(NTT):
                    for k in range(2):
                        nc.gpsimd.indirect_dma_start(
                            out=inv_rows[:, :], out_offset=IndirectOffsetOnAxis(
                                ap=off2_all[:, tt, k:k + 1], axis=0),
                            in_=tok16_t[:, tt, :], in_offset=None)
                # wrapped int16 idx tiles: partition r*16+s, col (e, c) =
                # token of slot c*16+s (replicated for the Q7 cores); on the
                # gpsimd queue so the waits don't block sync weight loads
                for r in range(8):
                    nc.gpsimd.dma_start(out=it_all[r * 16:(r + 1) * 16, :, :],
                                        in_=inv16.transpose((2, 3, 0, 1))[:, r])

            it_all = cpool.tile([128, E, NIC], I16, tag="idx")

            # ---- Shared MLP GEMM1 into resident gs_full -------------------
            # (PE streams this while the softmax/positions/scatter/gather
            # routing chain runs on the other engines)
            gs_full = bpool.tile([128, NHC, TC], DT, tag="gshared")
            for hqg in range(8):
                if hqg == 0:
                    wq1, wq3 = sw1_0, sw3_0
                else:
                    wq1 = wpool.tile([128, NDC, 256], DT, tag="w1q", bufs=3)
                    nc.sync.dma_start(out=wq1[:], in_=ws1p[hqg])
                    wq3 = wpool.tile([128, NDC, 256], DT, tag="w3q", bufs=3)
                    nc.sync.dma_start(out=wq3[:], in_=ws3p[hqg])
                for ht in range(2):
                    hg = hqg * 2 + ht
                    for ts in range(2):
                        psu = ps_uv.tile([128, 512], F32, tag="psu")
                        psv = ps_uv.tile([128, 512], F32, tag="psv")
                        for dc in range(NDC):
                            nc.tensor.matmul(
                                psu[:],
                                wq1[:, dc, ht * 128:(ht + 1) * 128],
                                xtr_t[:, dc, ts * 512:(ts + 1) * 512],
                                start=(dc == 0), stop=(dc == NDC - 1))
                        for dc in range(NDC):
                            nc.tensor.matmul(
                                psv[:],
                                wq3[:, dc, ht * 128:(ht + 1) * 128],
                                xtr_t[:, dc, ts * 512:(ts + 1) * 512],
                                start=(dc == 0), stop=(dc == NDC - 1))
                        su = kpool.tile([128, 512], F32, tag="su")
                        nc.scalar.activation(su[:], psu[:], AF.Silu)
                        nc.vector.tensor_mul(
                            gs_full[:, hg, ts * 512:(ts + 1) * 512],
                            su[:], psv[:])
                if hqg == 0:
                    # mask_all is ready by now; run the routing chain so the
                    # expert gathers complete long before the expert GEMMs
                    emit_positions_and_dispatch()

            out_v = out.rearrange("(tt p) d -> p tt d", p=128)

            # shared GEMM2 weights for the fused combine
            w2s_tiles = [wpool.tile([128, NHC, DW], DT, tag="w2s", bufs=2,
                                    name=f"w2s_{dq}") for dq in range(NDQ)]

            # ---- Experts: two halves of 4 ---------------------------------
            EH = E // 2
            for half in range(2):
                g_all = bpool.tile([128, EH, NHC, CAPMAX], DT, tag="g",
                                   name=f"g_all_{half}")
                for ei in range(EH):
                    e = half * EH + ei
                    ce = caps[e]
                    xe_t = kpool.tile([128, NDC, CAPPAD], DT, tag="xe", bufs=2)
                    nc.gpsimd.dma_gather(
                        out_ap=xe_t[:], in_ap=x_tok[:, :],
                        idxs_ap=it_all[:, e, :],
                        num_idxs=CAPPAD, num_idxs_reg=CAPPAD,
                        elem_size=D, transpose=True)

                    for hq in range(8):
                        wq1 = wpool.tile([128, NDC, 256], DT, tag="w1q", bufs=3)
                        nc.sync.dma_start(out=wq1[:], in_=w1p[e, hq])
                        wq3 = wpool.tile([128, NDC, 256], DT, tag="w3q", bufs=3)
                        nc.sync.dma_start(out=wq3[:], in_=w3p[e, hq])
                        for ht in range(2):
                            hg = hq * 2 + ht
                            psu = ps_uv.tile([128, CAPMAX], F32, tag="psu")
                            psv = ps_uv.tile([128, CAPMAX], F32, tag="psv")
                            for dc in range(NDC):
                                nc.tensor.matmul(
                                    psu[:, :ce],
                                    wq1[:, dc, ht * 128:(ht + 1) * 128],
                                    xe_t[:, dc, :ce],
                                    start=(dc == 0), stop=(dc == NDC - 1))
                            for dc in range(NDC):
                                nc.tensor.matmul(
                                    psv[:, :ce],
                                    wq3[:, dc, ht * 128:(ht + 1) * 128],
                                    xe_t[:, dc, :ce],
                                    start=(dc == 0), stop=(dc == NDC - 1))
                            su = kpool.tile([128, CAPMAX], F32, tag="su")
                            nc.scalar.activation(su[:, :ce], psu[:, :ce], AF.Silu)
                            nc.vector.tensor_mul(g_all[:, ei, hg, :ce],
                                                 su[:, :ce], psv[:, :ce])

                # prefetch shared GEMM2 weights while expert GEMM2 runs
                if half == 1:
                    nc.scalar.dma_start(out=w2s_tiles[0][:], in_=ws2p[0])

                # GEMM2 for this half's 4 experts, d-half (dq) outer
                for dq in range(NDQ):
                    for ei in range(EH):
                        e = half * EH + ei
                        ce = caps[e]
                        nct = (ce + 127) // 128
                        psy_l = [ps_y.tile([128, DW], F32, tag="psy",
                                           name=f"psy_{e}_{dq}_{i}")
                                 for i in range(nct)]
                        for qh in range(2):
                            w2q = wpool.tile([128, 8, DW], DT, tag="w2q")
                            nc.sync.dma_start(out=w2q[:], in_=w2p[e, dq, qh])
                            for ct in range(nct):
                                cw = min(128, ce - ct * 128)
                                for hc in range(8):
                                    nc.tensor.matmul(
                                        psy_l[ct][:cw],
                                        g_all[:, ei, qh * 8 + hc,
                                              ct * 128:ct * 128 + cw],
                                        w2q[:, hc, :],
                                        start=(qh == 0 and hc == 0),
                                        stop=(qh == 1 and hc == 7))
                        for ct in range(nct):
                            cw = min(128, ce - ct * 128)
                            ysb = kpool.tile([128, DW], F32, tag="ysb")
                            nc.vector.tensor_copy(ysb[:cw], psy_l[ct][:cw])
                            nc.scalar.dma_start(
                                out=ybufs[dq][ybase[e] + ct * 128:
                                              ybase[e] + ct * 128 + cw, :],
                                in_=ysb[:cw])

            # ---- Combine fused with shared GEMM2 --------------------------
            # PE computes the shared-MLP contribution per (dq, tt) while the
            # indirect gathers of the two expert rows run on the DMA engines.
            for dq in range(NDQ):
                if dq + 1 < NDQ:
                    nc.scalar.dma_start(out=w2s_tiles[dq + 1][:],
                                        in_=ws2p[dq + 1])
                w2s = w2s_tiles[dq]
                for tt in range(NTT):
                    psy = ps_y.tile([128, DW], F32, tag="psy")
                    for hc in range(NHC):
                        nc.tensor.matmul(
                            psy[:],
                            gs_full[:, hc, tt * 128:(tt + 1) * 128],
                            w2s[:, hc, :],
                            start=(hc == 0), stop=(hc == NHC - 1))
                    y1 = kpool.tile([128, DW], F32, tag="late", bufs=3)
                    nc.gpsimd.indirect_dma_start(
                        out=y1[:], out_offset=None,
                        in_=ybufs[dq][:, :],
                        in_offset=IndirectOffsetOnAxis(
                            ap=off_all[:, tt, 0:1], axis=0))
                    y2 = kpool.tile([128, DW], F32, tag="late2", bufs=3)
                    nc.gpsimd.indirect_dma_start(
                        out=y2[:], out_offset=None,
                        in_=ybufs[dq][:, :],
                        in_offset=IndirectOffsetOnAxis(
                            ap=off_all[:, tt, 1:2], axis=0))
                    fin = kpool.tile([128, DW], F32, tag="fin", bufs=3)
                    nc.vector.tensor_scalar_mul(
                        fin[:], y1[:], scalar1=t8_all[:, tt, 0:1])
                    y2s = kpool.tile([128, DW], F32, tag="y2s", bufs=3)
                    nc.scalar.activation(y2s[:], y2[:], AF.Copy,
                                         scale=t8_all[:, tt, 1:2])
                    nc.vector.tensor_add(fin[:], fin[:], y2s[:])
                    nc.vector.tensor_add(fin[:], fin[:], psy[:])
                    nc.scalar.dma_start(
                        out=out_v[:, tt, dq * DW:(dq + 1) * DW],
                        in_=fin[:])

    nc.finalize()
    return nc


def _get_program(caps):
    key = tuple(caps)
    if key not in _PROGRAMS:
        _PROGRAMS[key] = _build_program(key)
    return _PROGRAMS[key]


def _pack_w13(w):
    # [E, D, HID] -> [E, hq, p, dc, col] so each (e,hq) load is contiguous
    return np.ascontiguousarray(
        w.reshape(E, NDC, 128, 8, 256).transpose(0, 3, 2, 1, 4).astype(NP_DT))


def _pack_w2(w):
    # [E, HID, D] -> [E, dq, qh, p, hcl, col]
    return np.ascontiguousarray(
        w.reshape(E, 2, 8, 128, NDQ, DW).transpose(0, 4, 1, 3, 2, 5).astype(NP_DT))


def _pack_ws13(w):
    # [D, SH] -> [hqg, p, dc, col]
    return np.ascontiguousarray(
        w.reshape(NDC, 128, 8, 256).transpose(2, 1, 0, 3).astype(NP_DT))


def _pack_ws2(w):
    # [SH, D] -> [dq, p, hc, col]
    return np.ascontiguousarray(
        w.reshape(NHC, 128, NDQ, DW).transpose(2, 1, 0, 3).astype(NP_DT))


def _routing_caps(xf, w_router):
    """Per-expert capacity: max per-core count from fp32 routing + slack."""
    logits = xf @ w_router
    part = np.argpartition(-logits, 2, axis=1)[:, :2]
    mask = np.zeros(logits.shape, bool)
    mask[np.arange(len(xf))[:, None], part] = True
    cnt = mask.reshape(NCORES, TC, E).sum(axis=1)
    return [int(c) + CSLACK for c in cnt.max(axis=0)]


def kernel(x, w_router, w1, w3, w2, ws1, ws3, ws2):
    x = np.asarray(x, dtype=np.float32)
    w_router = np.ascontiguousarray(np.asarray(w_router, dtype=np.float32))
    w1 = np.asarray(w1, dtype=np.float32)
    w3 = np.asarray(w3, dtype=np.float32)
    w2 = np.asarray(w2, dtype=np.float32) * (2.0 / 3.0)
    ws1 = np.asarray(ws1, dtype=np.float32)
    ws3 = np.asarray(ws3, dtype=np.float32)
    ws2 = np.asarray(ws2, dtype=np.float32) * (1.0 / 3.0)

    orig_shape = x.shape
    xf = np.ascontiguousarray(x.reshape(T, D))

    caps = _routing_caps(xf, w_router)
    ybase = np.concatenate([[0], np.cumsum(caps)[:-1]]).astype(np.float32)

    idx = np.arange(128, dtype=np.float32)
    uts = (idx[:, None] < idx[None, :]).astype(np.float32)
    ones = np.ones((128, 128), dtype=np.float32)
    ecap = np.broadcast_to(ybase, (128, E))
    ecap2 = np.broadcast_to(
        np.arange(E, dtype=np.float32) * CAPPAD, (128, E))
    cpack = np.ascontiguousarray(
        np.concatenate([uts, ones, ecap, ecap2], axis=1, dtype=np.float32))
    tok = (np.arange(TC, dtype=np.int16).reshape(NTT, 128).T)[:, :, None]
    tok16 = np.ascontiguousarray(np.broadcast_to(tok, (128, NTT, 16)))

    w1p, w3p = _pack_w13(w1), _pack_w13(w3)
    w2p = _pack_w2(w2)
    ws1p, ws3p = _pack_ws13(ws1), _pack_ws13(ws3)
    ws2p = _pack_ws2(ws2)
    wrp_h = np.ascontiguousarray(
        w_router.reshape(NDC, 128, E).transpose(1, 0, 2).astype(NP_DT))

    nc = _get_program(caps)

    in_maps = []
    for c in range(NCORES):
        xc = np.ascontiguousarray(xf[c * TC:(c + 1) * TC])
        xtrp = np.ascontiguousarray(
            xc.T.reshape(NDC, 128, TC).transpose(1, 0, 2).astype(NP_DT))
        in_maps.append({
            "x_tok": xc.astype(NP_DT), "x_trp": xtrp,
            "wrp": wrp_h,
            "w1p": w1p, "w3p": w3p, "w2p": w2p,
            "ws1p": ws1p, "ws3p": ws3p, "ws2p": ws2p,
            "cpack": cpack, "tok16": tok16,
        })

    res = run_bass_kernel_spmd(nc, in_maps, list(range(NCORES)))
    out = np.concatenate([res.results[c]["out"] for c in range(NCORES)], axis=0)
    return out.reshape(orig_shape).astype(np.float32)
